# revision 6
# baseline (speedup 1.0000x reference)
"""Trainium2 Bass kernel for the Koopman Q-network problem.

Algebra: the reference
    phi_x = state @ W_phi.T            [B, 128]
    psi_u = action @ W_psi.T           [B, 8]
    ephi  = einsum('bj,ijz,bz->bi', phi_x, K, psi_u)
    out   = ephi @ w_lin.T             [B, 1]
collapses (fold the parameter-only contractions) to a per-sample bilinear form
    out[b] = state[b]^T @ M @ action[b],   M = W_phi.T @ (sum_i w_lin[i] K[i]) @ W_psi   [17, 7]

Device kernel (per core, 16384 samples), data-parallel over 8 cores:
  - host packs a transposed joint layout v[128, 4096]: partition 32c+q holds
    (q=0..6: action dims, q=7..23: state dims, q=24..31: zero pad) for the
    4096 samples of chunk c
  - stage 1: one full-width matmul per 512-col group with a block-diagonal
    stationary weight WV[128,128] (4 copies of the padded M on the diagonal)
    -> t = state @ M lands in PSUM on the same partitions as the action rows
  - stage 2: DVE elementwise multiply prod = t_psum * v
  - stage 3: selector matmul (ones at the action rows, output row 4j+c)
    accumulates the partition-reduce for all 8 groups into one PSUM bank
  - copy PSUM -> SBUF, one 64KB DMA out per core
"""

import numpy as np

import concourse.bacc as bacc
import concourse.tile as tile
from concourse import mybir
from concourse.bass_utils import run_bass_kernel_spmd

B = 131072
SD = 17  # state dim
AD = 7   # action dim
NCORES = 8
PB = B // NCORES          # 16384 samples per core
NBLK = 4                  # partition blocks of 32
CHUNK = PB // NBLK        # 4096 samples per block
GW = 512                  # matmul moving free dim
NG = CHUNK // GW          # 8 col groups
NDMA = 2                  # input DMA split for load/compute overlap

_f32 = mybir.dt.float32
_nc_cache = {}


CONST_W = 128 + NG * 32      # wv cols + sel cols packed ahead of data
HW_COLS = CHUNK // NDMA      # data cols per DMA half


def _build_nc():
    nc = bacc.Bacc("TRN2", target_bir_lowering=False, debug=False)
    # d0 packs [wv | sel | first data half] so the first matmul waits on a
    # single DMA semaphore (LDWEIGHTS only has one sync-wait slot).
    d0 = nc.dram_tensor("d0", [128, CONST_W + HW_COLS], _f32, kind="ExternalInput")
    d1 = nc.dram_tensor("d1", [128, HW_COLS], _f32, kind="ExternalInput")
    o = nc.dram_tensor("o", [NG * NBLK, GW], _f32, kind="ExternalOutput")

    with tile.TileContext(nc) as tc:
        with (
            tc.tile_pool(name="data", bufs=1) as data,
            tc.tile_pool(name="prodp", bufs=4) as prodp,
            tc.tile_pool(name="tpsp", bufs=4, space="PSUM") as tpsp,
            tc.tile_pool(name="opsp", bufs=1, space="PSUM") as opsp,
            tc.tile_pool(name="outp", bufs=1) as outp,
        ):
            big0 = data.tile([128, CONST_W + HW_COLS], _f32)
            nc.sync.dma_start(out=big0[:, :], in_=d0[:, :])
            big1 = data.tile([128, HW_COLS], _f32)
            nc.sync.dma_start(out=big1[:, :], in_=d1[:, :])

            wv_sb = big0[:, 0:128]
            sel_sb = big0[:, 128:CONST_W]
            halves = [big0[:, CONST_W:], big1[:, :]]

            o_ps = opsp.tile([NG * NBLK, GW], _f32)
            gph = NG // NDMA
            for j in range(NG):
                vslice = halves[j // gph][:, (j % gph) * GW:(j % gph) * GW + GW]
                t_ps = tpsp.tile([128, GW], _f32, name="t_ps")
                nc.tensor.matmul(t_ps[:, :], lhsT=wv_sb, rhs=vslice,
                                 start=True, stop=True)
                prod = prodp.tile([128, GW], _f32, name="prod")
                nc.vector.tensor_mul(prod[:, :], t_ps[:, :], vslice)
                nc.tensor.matmul(o_ps[:, :], lhsT=sel_sb[:, j * 32:(j + 1) * 32],
                                 rhs=prod[:, :], start=(j == 0), stop=(j == NG - 1),
                                 skip_group_check=True)

            o_sb = outp.tile([NG * NBLK, GW], _f32)
            nc.vector.tensor_copy(o_sb[:, :], o_ps[:, :])
            nc.sync.dma_start(out=o[:, :], in_=o_sb[:, :])
    nc.compile()
    return nc


def _get_nc():
    if "nc" not in _nc_cache:
        _nc_cache["nc"] = _build_nc()
    return _nc_cache["nc"]


def _fold_M(W_phi, W_psi, K, w_lin):
    A = np.einsum("i,ijz->jz", w_lin[0].astype(np.float64), K.astype(np.float64))
    return (W_phi.astype(np.float64).T @ A @ W_psi.astype(np.float64)).astype(np.float32)


def _make_weights(M):
    wq = np.zeros((32, 32), np.float32)
    wq[AD:AD + SD, 0:AD] = M
    WV = np.zeros((128, 128), np.float32)
    for c in range(NBLK):
        WV[32 * c:32 * c + 32, 32 * c:32 * c + 32] = wq
    SEL = np.zeros((128, NG, 32), np.float32)
    for c in range(NBLK):
        for j in range(NG):
            SEL[32 * c:32 * c + AD, j, NBLK * j + c] = 1.0
    return WV, SEL.reshape(128, NG * 32)


def _make_in_maps(state, action, WV, SEL):
    st = state.reshape(NCORES, NBLK, CHUNK, SD)
    ac = action.reshape(NCORES, NBLK, CHUNK, AD)
    in_maps = []
    for i in range(NCORES):
        v = np.zeros((NBLK, 32, CHUNK), np.float32)
        v[:, 0:AD] = ac[i].transpose(0, 2, 1)
        v[:, AD:AD + SD] = st[i].transpose(0, 2, 1)
        v = v.reshape(128, CHUNK)
        d0 = np.ascontiguousarray(
            np.concatenate([WV, SEL, v[:, :HW_COLS]], axis=1))
        d1 = np.ascontiguousarray(v[:, HW_COLS:])
        in_maps.append({"d0": d0, "d1": d1})
    return in_maps


def _unscramble(o):
    # o[4j+c, n] = out[c*CHUNK + j*GW + n]
    return o.reshape(NG, NBLK, GW).transpose(1, 0, 2).reshape(PB)


def _run(inputs, trace=False):
    state = np.ascontiguousarray(np.asarray(inputs["state"], dtype=np.float32))
    action = np.ascontiguousarray(np.asarray(inputs["action"], dtype=np.float32))
    M = _fold_M(np.asarray(inputs["W_phi"]), np.asarray(inputs["W_psi"]),
                np.asarray(inputs["K"]), np.asarray(inputs["w_lin"]))
    WV, SEL = _make_weights(M)
    in_maps = _make_in_maps(state, action, WV, SEL)
    res = run_bass_kernel_spmd(_get_nc(), in_maps, core_ids=list(range(NCORES)),
                               trace=trace)
    out = np.concatenate([_unscramble(res.results[i]["o"]) for i in range(NCORES)])
    return out.reshape(B, 1).astype(np.float32), res


def kernel(state, action, W_phi, W_psi, K, w_lin):
    out, _ = _run({"state": state, "action": action, "W_phi": W_phi,
                   "W_psi": W_psi, "K": K, "w_lin": w_lin})
    return out
